# revision 9
# baseline (speedup 1.0000x reference)
"""BERT self-attention (B=16, T=512, C=768, H=12, D=64) on 8 trn2 NeuronCores.

Data-parallel over batch: each core gets 2 batches. Matmul operands fp16,
fp32 PSUM accumulation. Per core:
  xT     shipped pre-transposed from host ([C, M] per core) -> no on-chip
         transposes; W_qk shipped n-major so the first QK matmul chain can
         start as soon as one 196KB slice + xT-b0 land (~3us).
  warmup zero-matmuls run during the input DMA window so HAM un-throttles
         the PE clock before real matmuls start.
  Q^T/K^T [feature, token] (lhsT = wqk tile), V [token, feature] in
         compact 65-col per-head blocks [V_h | ones] (lhsT = xT tile).
  S^T    = K^T-as-lhsT matmul -> [key, query]; two heads of a pair go to
         separate PSUM banks at row positions 0/64 (row-split tile packing).
  P      = exp(S/8 + mask) on ScalarE, fp16. ScalarE runs ONLY Exp and
         table-free Identity/Copy -> zero ACT_TABLE_LOAD thrash.
  y^T    = lhsT=[V_h | ones] matmul -> unnormalized y^T + row sums in PSUM
         row 64; reciprocal_approx_fast on VectorE reads the PSUM row
         directly; GpSimd casts recips to fp16; a K=2 matmul against a 0/1
         pattern replicates the two recip rows across 128 partitions;
         normalization multiply on VectorE.
  out    = y^T-as-lhsT matmul + bias add -> fp16 staged tile, DMA out.
Bias replication via 0-stride broadcast DMA; biases shipped f32 from host.
"""

import sys

sys.path.insert(0, "/opt/trn_rl_repo")

from contextlib import ExitStack

import numpy as np

B, T, C = 16, 512, 768
H, D = 12, 64
C3 = 3 * C
N_CORES = 8
BC = B // N_CORES           # batches per core
M = BC * T                  # tokens per core
KT = C // 128               # feature k-tiles (6)
TT = M // 128               # token tiles per core (8)
NQK = 2 * C // 128          # q+k feature n-tiles (12)
VW = H * 65                 # v tile width: per-head [V_h | ones] blocks
SCALE = 1.0 / np.sqrt(D)

_cache = {}


def _build():
    import concourse.bass as bass
    import concourse.tile as tile
    from concourse import bacc, mybir
    f32 = mybir.dt.float32
    f16 = mybir.dt.float16
    Exp = mybir.ActivationFunctionType.Exp
    Ident = mybir.ActivationFunctionType.Identity
    Add = mybir.AluOpType.add
    Mult = mybir.AluOpType.mult

    nc = bacc.Bacc("TRN2", target_bir_lowering=False, debug=False,
                   num_devices=N_CORES)
    xt_d = nc.dram_tensor("xt", [C, M], f16, kind="ExternalInput").ap()
    wqk_d = nc.dram_tensor("wqk", [NQK * C, 128], f16,
                           kind="ExternalInput").ap()
    wv_d = nc.dram_tensor("wv", [C, C], f16, kind="ExternalInput").ap()
    # qm32: cols 0-11 = ba_qk per-partition, cols 12-19 = mask per-partition
    qm_d = nc.dram_tensor("qm32", [128, 20], f32, kind="ExternalInput").ap()
    # bb32: row of [ba_v (768) | b_proj (768)], broadcast to 128 partitions
    bb_d = nc.dram_tensor("bb32", [1, 2 * C], f32, kind="ExternalInput").ap()
    wp_d = nc.dram_tensor("w_proj", [C, C], f16, kind="ExternalInput").ap()
    e2_d = nc.dram_tensor("e2pat", [2, 128], f16, kind="ExternalInput").ap()
    out_d = nc.dram_tensor("out", [M, C], f16, kind="ExternalOutput").ap()

    with tile.TileContext(nc) as tc, ExitStack() as ctx:
        pp = ctx.enter_context(tc.tile_pool(name="pp", bufs=1))
        np_ = ctx.enter_context(tc.tile_pool(name="norm", bufs=4))
        ap_ = ctx.enter_context(tc.tile_pool(name="att", bufs=6))
        ps_mm = ctx.enter_context(tc.tile_pool(name="ps_mm", bufs=2, space="PSUM"))
        ps_s = ctx.enter_context(tc.tile_pool(name="ps_s", bufs=2, space="PSUM"))
        ps_y = ctx.enter_context(tc.tile_pool(name="ps_y", bufs=2, space="PSUM"))

        # ---- static tiles ----
        qm = pp.tile([128, 20], f32, tag="qm")
        ba_qk = qm[:, 0:NQK]
        mask_sb = qm[:, NQK:NQK + BC * 4]
        bb = pp.tile([128, 2 * C], f32, tag="bb")
        ba_v_rep = bb[:, 0:C]
        bp_rep = bb[:, C:2 * C]
        e2a = pp.tile([1, 128], f16, tag="e2a")
        e2b = pp.tile([1, 128], f16, tag="e2b")
        wqk_t = pp.tile([128, NQK, KT, 128], f16, tag="wqk")
        wv_t = pp.tile([128, KT, C], f16, tag="wv")
        wp_all = pp.tile([128, KT, C], f16, tag="wp")
        wp_t = [wp_all[:, k, :] for k in range(KT)]
        xt_t = pp.tile([128, KT, M], f16, tag="xT")
        v_t = [pp.tile([128, VW], f16, tag=f"v{t}", name=f"v{t}")
               for t in range(TT)]
        warm = pp.tile([128, 512], f16, tag="warm")

        # ---- PE warmup during the input-DMA window (HAM un-throttle) ----
        nc.vector.memset(warm[:], 0.0)
        for i in range(8):
            pw = ps_mm.tile([128, 512], f32, tag="mm", name=f"warm{i}")
            nc.tensor.matmul(pw[:], warm[:, 0:128], warm[:],
                             start=True, stop=True)
        # Pool compute early so nothing queues behind later work
        for t in range(TT):
            nc.gpsimd.memset(
                v_t[t].rearrange("p (h c) -> p h c", c=65)[:, :, 64:65],
                1.0)

        # ---- input DMAs ----
        # sync ring: xT halves (b0 first), then wv, wp
        nc.sync.dma_start(
            xt_t[:, 0:3, 0:T],
            xt_d[0:384, 0:T].rearrange("(k p) m -> p k m", p=128))
        nc.sync.dma_start(
            xt_t[:, 3:KT, 0:T],
            xt_d[384:C, 0:T].rearrange("(k p) m -> p k m", p=128))
        nc.sync.dma_start(
            xt_t[:, 0:3, T:M],
            xt_d[0:384, T:M].rearrange("(k p) m -> p k m", p=128))
        nc.sync.dma_start(
            xt_t[:, 3:KT, T:M],
            xt_d[384:C, T:M].rearrange("(k p) m -> p k m", p=128))
        nc.sync.dma_start(
            wv_t[:, 0:3, :],
            wv_d[0:384, :].rearrange("(k p) c -> p k c", p=128))
        nc.sync.dma_start(
            wv_t[:, 3:KT, :],
            wv_d[384:C, :].rearrange("(k p) c -> p k c", p=128))
        nc.sync.dma_start(
            wp_all[:],
            wp_d[:, :].rearrange("(k p) c -> p k c", p=128))
        # scalar ring: small tiles + n-major wqk slices
        nc.scalar.dma_start(qm[:], qm_d[:])
        for n in range(NQK):
            nc.scalar.dma_start(
                wqk_t[:, n, :, :],
                wqk_d[n * C:(n + 1) * C, :].rearrange(
                    "(k p) c -> p k c", p=128))
            if n == 5:
                nc.scalar.dma_start(
                    bb[:], bb_d[0:1, :].partition_broadcast(128))
                nc.scalar.dma_start(e2a[:], e2_d[0:1, :])
                nc.scalar.dma_start(e2b[:], e2_d[1:2, :])

        ones_r = pp.tile([1, 128], f16, tag="ones_r")
        nc.vector.memset(ones_r[:], 1.0)
        bp16 = pp.tile([1, C], f16, tag="bp16")
        nc.vector.tensor_copy(bp16[:], bb[0:1, C:2 * C])

        qkT = [pp.tile([128, M], f16, tag=f"qk{n}", name=f"qk{n}")
               for n in range(NQK)]
        yT_t = [pp.tile([128, M], f16, tag=f"yT{c}", name=f"yT{c}")
                for c in range(KT)]

        def qkv_chain(b, i):
            """i in [0, 20): 12 QK n-tiles then 8 V half-tiles."""
            bcol = b * T
            if i < NQK:
                n = i
                p = ps_mm.tile([128, 512], f32, tag="mm", name=f"mm{b}_{i}")
                for k in range(KT):
                    nc.tensor.matmul(
                        p[:],
                        wqk_t[:, n, k, :],
                        xt_t[:, k, bcol:bcol + T],
                        start=(k == 0), stop=(k == KT - 1))
                if b == 0:
                    # ScalarE Identity-with-bias: table-free, unloads DVE
                    nc.scalar.activation(
                        qkT[n][:, bcol:bcol + T], p[:], Ident,
                        bias=ba_qk[:, n:n + 1])
                else:
                    nc.vector.tensor_scalar_add(
                        qkT[n][:, bcol:bcol + T], p[:], ba_qk[:, n:n + 1])
            else:
                j = i - NQK
                t = b * 4 + j // 2
                lo, w = ((0, 512), (512, 256))[j % 2]
                p = ps_mm.tile([128, 512], f32, tag="mm", name=f"mm{b}_{i}")
                for k in range(KT):
                    nc.tensor.matmul(
                        p[:, :w],
                        xt_t[:, k, t * 128:(t + 1) * 128],
                        wv_t[:, k, lo:lo + w],
                        start=(k == 0), stop=(k == KT - 1))
                h0 = lo // D
                nc.vector.tensor_tensor(
                    out=v_t[t].rearrange("p (h c) -> p h c", c=65)
                        [:, h0:h0 + w // D, 0:64],
                    in0=p[:, :w].rearrange("p (h c) -> p h c", c=D),
                    in1=ba_v_rep[:, lo:lo + w].rearrange(
                        "p (h c) -> p h c", c=D),
                    op=Add)

        py_tiles = {}
        pair_tiles = {}
        srow_tiles = {}

        def attention_hp(b, hp):
            bcol = b * T
            # rs row 64: [sums_h0 | sums_h1] staged fp16 (same-base copy)
            rs = np_.tile([65, 1024], f16, tag="rstage", bufs=4,
                          name=f"rs{b}_{hp}")
            srow = np_.tile([1, 1024], f16, tag="srow", bufs=4,
                            name=f"srow{b}_{hp}")
            srow_tiles[(b, hp)] = srow
            pair = np_.tile([128, 512], f16, tag="pair", bufs=4,
                            name=f"pair{b}_{hp}")
            pair_tiles[(b, hp)] = pair
            e_tiles = []
            for kt in range(4):
                ps = ps_s.tile([128, 1024], f32)
                for sub in range(2):
                    r0 = 64 * sub
                    nc.tensor.matmul(
                        ps[:, sub * 512:sub * 512 + 512],
                        qkT[6 + hp][r0:r0 + D,
                                    bcol + kt * 128:bcol + (kt + 1) * 128],
                        qkT[hp][r0:r0 + D, bcol:bcol + T],
                        start=True, stop=True)
                e = ap_.tile([128, 1024], f16, tag="e")
                nc.scalar.activation(
                    e[:], ps[:], Exp,
                    bias=mask_sb[:, b * 4 + kt:b * 4 + kt + 1],
                    scale=float(SCALE))
                e_tiles.append(e)
            for sub in range(2):
                h = 2 * hp + sub
                py = ps_y.tile([128, 512], f32)
                py_tiles[(b, hp, sub)] = py
                for kt in range(4):
                    nc.tensor.matmul(
                        py[0:65, :],
                        v_t[b * 4 + kt][:, 65 * h:65 * (h + 1)],
                        e_tiles[kt][:, sub * 512:sub * 512 + 512],
                        start=(kt == 0), stop=(kt == 3))
                # stage softmax denominators (same-base copy, fp16 cast)
                nc.scalar.copy(
                    rs[64:65, sub * 512:sub * 512 + 512], py[64:65, :])
                if sub == 1:
                    # sub1's features must move to partitions 64-127: copy
                    # then DMA-relocate (partitions are DMA-only territory)
                    st = np_.tile([64, 512], f16, tag="stage")
                    nc.vector.tensor_copy(st[:], py[0:64, :])
                    nc.sync.dma_start(pair[64:128, :], st[:])
            # relocate the sums row to partition 0
            nc.sync.dma_start(srow[:], rs[64:65, :])

        def norm_apply(b, hp, tc_lo=0, tc_hi=4):
            """Normalize head-pair hp (k-tile hp) of batch b, token chunks
            [tc_lo, tc_hi) of 128 cols each."""
            bcol = b * T
            srow = srow_tiles[(b, hp)]
            if tc_lo == 0:
                # replicate sums across partitions: h0 -> 0-63, h1 -> 64-127
                rep = ps_mm.tile([128, 512], f32, tag="mm",
                                 name=f"rep{b}_{hp}")
                nc.tensor.matmul(
                    rep[:], e2a[:], srow[0:1, 0:512], start=True, stop=False)
                nc.tensor.matmul(
                    rep[:], e2b[:], srow[0:1, 512:1024], start=False,
                    stop=True)
                rcp = np_.tile([128, 512], f32, tag="rcp", bufs=3,
                               name=f"rcp{b}_{hp}")
                srow_tiles[(b, hp, "rcp")] = rcp
                nc.vector.reciprocal_approx_fast(rcp[:], rep[:])
            else:
                rcp = srow_tiles[(b, hp, "rcp")]
            py0 = py_tiles[(b, hp, 0)]
            pair = pair_tiles[(b, hp)]
            for tc in range(tc_lo, tc_hi):
                lo = tc * 128
                nc.vector.tensor_tensor(
                    out=yT_t[hp][0:64, bcol + lo:bcol + lo + 128],
                    in0=py0[0:64, lo:lo + 128],
                    in1=rcp[0:64, lo:lo + 128],
                    op=Mult)
                nc.gpsimd.tensor_tensor(
                    out=yT_t[hp][64:128, bcol + lo:bcol + lo + 128],
                    in0=pair[64:128, lo:lo + 128],
                    in1=rcp[64:128, lo:lo + 128],
                    op=Mult)

        pj_part = {}
        ot_tiles = {}

        def proj_chunk(b, i, ks=0, ke=KT, partial=False, tail=False):
            t = b * 4 + i // 2
            lo, w = ((0, 512), (512, 256))[i % 2]
            p = ps_mm.tile([128, 512], f32, tag="mm", name=f"pj{b}_{i}_{ks}")
            for k in range(ks, ke):
                nc.tensor.matmul(
                    p[:, :w],
                    yT_t[k][:, t * 128:(t + 1) * 128],
                    wp_t[k][:, lo:lo + w],
                    start=(k == ks), stop=(k == ke - 1 and not partial))
            if partial:
                # fold the bias in as a K=1 matmul; evacuate on ScalarE
                nc.tensor.matmul(
                    p[:, :w], ones_r[0:1, :], bp16[0:1, lo:lo + w],
                    start=False, stop=True)
                pt = np_.tile([128, 512], f32, tag="pjpart", bufs=8,
                              name=f"pjpart{i}")
                nc.scalar.copy(pt[:, :w], p[:, :w])
                pj_part[(b, i)] = pt
                return
            if tail:
                if i % 2 == 0:
                    ot = np_.tile([128, C], f16, tag="otail", bufs=4,
                                  name=f"ott{i}")
                    ot_tiles[(b, t)] = ot
                else:
                    ot = ot_tiles[(b, t)]
                off = lo
            else:
                if i % 4 == 0:
                    ot = np_.tile([128, 2 * C], f16, tag="ostage", bufs=3,
                                  name=f"ot{b}_{i}")
                    ot_tiles[(b, t // 2)] = ot
                else:
                    ot = ot_tiles[(b, t // 2)]
                off = (t % 2) * C + lo
            if (b, i) in pj_part:
                nc.vector.tensor_tensor(
                    out=ot[:, off:off + w], in0=p[:, :w],
                    in1=pj_part[(b, i)][:, :w], op=Add)
            else:
                nc.vector.tensor_tensor(
                    out=ot[:, off:off + w], in0=p[:, :w],
                    in1=bp_rep[:, lo:lo + w], op=Add)
            if tail and i % 2 == 1:
                q = nc.sync if (i // 2) % 2 == 0 else nc.scalar
                q.dma_start(out_d[t * 128:(t + 1) * 128, :], ot[:, :])
            elif not tail and i % 4 == 3:
                t0 = t - 1
                nc.sync.dma_start(
                    out_d[t0 * 128:(t0 + 2) * 128, :].rearrange(
                        "(t p) c -> p t c", p=128),
                    ot[:].rearrange("p (t c) -> p t c", c=C))

        # ---- software-pipelined emission ----
        CHAIN_ORDER = list(range(9)) + list(range(12, 20)) + [9, 10, 11]
        for i in CHAIN_ORDER:
            qkv_chain(0, i)
        qk1 = iter(CHAIN_ORDER)
        for hp in range(6):
            attention_hp(0, hp)
            if hp >= 1:
                norm_apply(0, hp - 1)
            for _ in range(4 if hp < 2 else 3):
                i = next(qk1, None)
                if i is not None:
                    qkv_chain(1, i)
        norm_apply(0, 5)
        pj0 = iter(range(8))
        for hp in range(6):
            attention_hp(1, hp)
            if hp >= 1:
                norm_apply(1, hp - 1)
            if hp == 5:
                for i in range(8):
                    proj_chunk(1, i, 0, 5, partial=True)
            for _ in range(1 if hp < 4 else 2):
                i = next(pj0, None)
                if i is not None:
                    proj_chunk(0, i)
        for i in pj0:
            proj_chunk(0, i)
        # pipelined tail: per 128-token chunk, normalize then project k=5
        for tc in range(4):
            norm_apply(1, 5, tc, tc + 1)
            proj_chunk(1, 2 * tc, 5, KT, tail=True)
            proj_chunk(1, 2 * tc + 1, 5, KT, tail=True)

    nc.compile()
    return nc


def get_compiled():
    if "nc" not in _cache:
        _cache["nc"] = _build()
    return _cache["nc"]


def make_in_maps(x, attention_mask, W_attn, b_attn, W_proj, b_proj):
    x = np.asarray(x, dtype=np.float32).astype(np.float16)
    mask = np.asarray(attention_mask, dtype=np.float32)[:, 0, 0, :]
    wa = np.asarray(W_attn, dtype=np.float32).astype(np.float16)
    ba = np.asarray(b_attn, dtype=np.float32)
    wp = np.asarray(W_proj, dtype=np.float32).astype(np.float16)
    bp = np.asarray(b_proj, dtype=np.float32)
    bb = np.ascontiguousarray(
        np.concatenate([ba[2 * C:], bp]).reshape(1, 2 * C))
    # n-major W_qk: for each 128-col n-block, its 6 k-slices contiguous
    wqk = np.ascontiguousarray(
        wa[:, :2 * C].reshape(C, NQK, 128).transpose(1, 0, 2).reshape(
            NQK * C, 128))
    wv = np.ascontiguousarray(wa[:, 2 * C:])
    e2 = np.zeros((2, 128), dtype=np.float16)
    e2[0, 0:64] = 1.0
    e2[1, 64:128] = 1.0
    maps = []
    for i in range(N_CORES):
        qm = np.zeros((128, 20), dtype=np.float32)
        qm[:, :NQK] = ba[0:2 * C].reshape(NQK, 128).T
        qm[:, NQK:] = mask[BC * i:BC * (i + 1)].reshape(-1).reshape(
            BC * 4, 128).T
        xt = np.ascontiguousarray(
            x[BC * i:BC * (i + 1)].reshape(M, C).T)
        maps.append({
            "xt": xt, "qm32": qm, "bb32": bb,
            "wqk": wqk, "wv": wv, "w_proj": wp, "e2pat": e2,
        })
    return maps


def kernel(x, attention_mask, W_attn, b_attn, W_proj, b_proj):
    from concourse.bass_utils import run_bass_kernel_spmd

    nc = get_compiled()
    in_maps = make_in_maps(x, attention_mask, W_attn, b_attn, W_proj, b_proj)
    last_err = None
    for _ in range(3):
        try:
            res = run_bass_kernel_spmd(nc, in_maps, list(range(N_CORES)))
            break
        except Exception as e:  # transient NRT device errors: retry
            last_err = e
    else:
        raise last_err
    out = np.concatenate(
        [res.results[i]["out"].reshape(BC, T, C) for i in range(N_CORES)],
        axis=0)
    return out.astype(np.float32)
